# revision 26
# baseline (speedup 1.0000x reference)
"""Trainium2 Bass kernel for nn_GatedLinear (gated LoRA-MoE linear layer).

Math (see reference):
  base_out = x @ base_w.T + base_b
  logits   = x @ router_w.T ; top-2 softmax -> dense per-expert gate
  h        = x @ lora_A.T   ; rank_w = repeat(gate*scalings, 16)
  out      = base_out + (h * rank_w) @ lora_B.T

Sharding: pure data-parallel over batch*seq across 8 cores (1024 tokens
per core); all weights replicated. No collectives.

Device-side strategy (v7):
  * x ships ONLY as an fp16 hi/lo split (xh + rh == x to ~2^-23 rel):
    16MB/core, the minimum for a top-2 selection that matches the fp32
    reference; partition-major layout so a 4-ko block is one 1MB DMA
    with 8KB contiguous runs. The fp8 copy for the base matmul is cast
    on device from xh by the scalar (ACT) engine -- 4MB less DMA in
    the phase-1 critical window than shipping it.
  * Router: the two xh terms (xh@rwh, xh@rwr) merge into ONE matmul
    with a 16-wide stationary [rwh|rwr]; the rh term (rh@rwh)
    accumulates into rows 0:8 of the same [16,GT] PSUM tile; the fold
    happens after the token-major transpose along the free axis (the
    BIR verifier rejects partition-offset PSUM reads).
  * Base matmul: fp8e4m3 DoubleRow (weights host-scaled x64, packed
    [k2,2,f]); measured 216ns per 256x128x512 step = the fp8 roofline
    (HAM-warm 2.4GHz, 1 cycle/moving-token). The x64 scale is folded
    into the e8 gate expansion and removed in the bias epilogue.
  * Four "early" base groups: ot=0 runs during phase-1 streaming
    (interleaved per-ko); ot=1 fills the PE while the DVE runs the
    gating chain (its PSUM banks come from the freed logits tiles).
    All four lora_B closes are emitted after the gating chain.
  * DMA queues: per-queue HBM share is ~1/3 of 358 GB/s when all three
    queues are loaded, so xh/rh blocks ALTERNATE between sync and
    gpsimd (neither stream bound by one queue's share); scalar's queue
    carries only small consts so its engine is free to cast; weight
    stream split scalar/gpsimd behind phase 1; fp16 outputs on sync.
  * Output is fp16 [O, tokens] per core (halves output DMA; ~5e-4 rel
    error) and de-transposed/cast to f32 on the host.

PSUM budget during phase 1 (8 banks): 2 logits + 2 h + 2 transpose
scratch + 2 early base groups; the logits banks recycle into 2 more
base groups mid-gating. Phase 2 uses 6 accumulation groups.
"""

from contextlib import ExitStack

import numpy as np


def _ensure_path():
    try:
        import concourse.bass  # noqa: F401
    except ImportError:
        import sys

        for p in ("/opt/trn_rl_repo", "/root/.axon_site/_ro/trn_rl_repo"):
            if p not in sys.path:
                sys.path.insert(0, p)


N_CORES = 8
B, S, D, O = 4, 2048, 4096, 4096
T = B * S              # 8192 tokens total
T_PC = T // N_CORES    # 1024 tokens per core
E = 8                  # experts
RANK = 16
R = E * RANK           # 128 fused rank dim
P = 128
KO = D // P            # 32 k-subtiles of the contraction dim
KO2 = KO // 2          # paired k-subtiles for DoubleRow (256-deep)
OTILES = O // P        # 32 output-feature tiles
TTILE = 512            # tokens per matmul moving operand
NT = T_PC // TTILE     # 2 token tiles per core
GT = 512               # gating token-tile size
NGT = T_PC // GT       # 2 gating tiles
NGC = GT // P          # 4 128-chunks per gating tile
W8_SCALE = 64.0        # base_w std is 1/64; scale into e4m3's sweet spot
FP8_BASE = True        # kept for test.py's sim threshold selection

_prog_cache = {}


def _build_program():
    """Build the single-core SPMD Bass program (same on all 8 cores)."""
    _ensure_path()
    import concourse.bass as bass
    import concourse.mybir as mybir
    import concourse.tile as tile
    from concourse import bacc

    f32 = mybir.dt.float32
    f16 = mybir.dt.float16
    bf16 = mybir.dt.bfloat16
    f8 = mybir.dt.float8e4
    Alu = mybir.AluOpType
    Act = mybir.ActivationFunctionType
    DR = mybir.MatmulPerfMode.DoubleRow

    nc = bacc.Bacc(
        "TRN2",
        target_bir_lowering=False,
        debug=False,
        num_devices=N_CORES,
    )

    # x hi/lo ship partition-major: a 4-ko block is one 1MB DMA with
    # 8KB contiguous runs. xh streams FIRST (router-hi + h + fp8 cast),
    # rh second (router-lo), so base-only matmul work can start as soon
    # as the fp8 casts exist.
    xh = nc.dram_tensor("xh", [P, KO * T_PC], f16, kind="ExternalInput").ap()
    rh = nc.dram_tensor("rh", [P, KO * T_PC], f16, kind="ExternalInput").ap()
    wt = nc.dram_tensor(
        "wt", [OTILES * P, KO2 * 2 * P], f8, kind="ExternalInput"
    ).ap()
    lb = nc.dram_tensor("lb", [P, O], bf16, kind="ExternalInput").ap()
    ar = nc.dram_tensor("ar", [P, KO * R], f16, kind="ExternalInput").ap()
    rw2 = nc.dram_tensor("rw2", [P, KO * 2 * E], f16, kind="ExternalInput").ap()
    bb = nc.dram_tensor("bb", [O], f32, kind="ExternalInput").ap()
    e8 = nc.dram_tensor("e8", [E, P], f32, kind="ExternalInput").ap()
    idm = nc.dram_tensor("idm", [P, P], f32, kind="ExternalInput").ap()
    yt = nc.dram_tensor("yt", [O, T_PC], f16, kind="ExternalOutput").ap()

    xh_v = xh.rearrange("p (ko t) -> p ko t", t=T_PC)
    rh_v = rh.rearrange("p (ko t) -> p ko t", t=T_PC)
    wt_v = wt.rearrange("(ot p) (k j f) -> p ot k j f", p=P, j=2, f=P)
    ar_v = ar.rearrange("p (ko r) -> p ko r", r=R)          # [128, 32, 128]
    rw2_v = rw2.rearrange("p (ko c) -> p ko c", c=2 * E)    # [128, 32, 16]
    bb_v = bb.rearrange("(ot p) -> p ot", p=P)              # [128, 32]
    yt_v = yt.rearrange("(ot p) t -> p ot t", p=P)          # [128, 32, 1024]

    # block schedule (same for xh then rh): small leads so the PE
    # starts early, then 1MB 4-ko blocks rotating over the 3 queues
    XBLOCKS = [(0, 1), (1, 1), (2, 2), (4, 4), (8, 4), (12, 4), (16, 4),
               (20, 4), (24, 4), (28, 4)]
    KOBLK = {}
    for bi, (s, n) in enumerate(XBLOCKS):
        for k in range(s, s + n):
            KOBLK[k] = (bi, k - s)

    N_SPLIT = 5   # ot tiles 0..4 run base-only early; lora added later

    with tile.TileContext(nc) as tc:
        with (
            tc.tile_pool(name="perm", bufs=1) as pp,
            tc.tile_pool(name="wstream", bufs=5) as wpool,
            tc.tile_pool(name="xring", bufs=4) as xpool,
            tc.tile_pool(name="rring", bufs=4) as rpool,
            tc.tile_pool(name="obuf", bufs=4) as ob,
            tc.tile_pool(name="obase", bufs=2 * N_SPLIT) as obb,
        ):
            # ---- consts ----
            rw2sb = pp.tile([P, KO, 2 * E], f16)
            nc.scalar.dma_start(rw2sb[:], rw2_v[:])
            w_sb = [None] * OTILES

            def wload(ot, eng):
                w_sb[ot] = wpool.tile(
                    [P, KO2, 2, P], f8, tag="w", name=f"w{ot}"
                )
                eng.dma_start(w_sb[ot][:], wt_v[:, ot, :, :, :])

            wload(0, nc.scalar)

            # resident tiles (x8 is the only resident copy of x)
            x8sb = pp.tile([P, KO, T_PC], f8)
            rgp = pp.tile([P, T_PC], bf16)   # per-rank gates [r, t]
            hwsb = pp.tile([P, T_PC], bf16)  # gated rank activations [r, t]
            lbsb = pp.tile([P, O], bf16)     # lora_B.T resident
            bbsb = pp.tile([P, OTILES], f32)
            arsb = pp.tile([P, KO, R], f16)
            e8sb = pp.tile([E, P], f32)
            idsb = pp.tile([P, P], f32)

            # DMA completions are tracked on 8 count-based semaphore
            # lanes assigned round-robin in EMISSION order; a consumer
            # waits for every earlier DMA sharing its lane. So: emit in
            # approximate completion order -- tiny consts + the first
            # weight tiles, then the x streams strictly rotating, then
            # the late-needed bulk.
            ARC = 8
            nc.gpsimd.dma_start(arsb[:, 0:ARC, :], ar_v[:, 0:ARC, :])
            wload(1, nc.gpsimd)

            # Queue rules learned from traces:
            #  * ring-slot DMAs stall their whole FIFO until the slot
            #    frees -> they live ONLY on sync/gpsimd (pure DMA
            #    engines) with 4-deep rings so waits release early;
            #  * the scalar engine runs the fp8 casts, so its queue
            #    gets only wait-free perm-tile DMAs;
            #  * HWDGE completion lanes are shared count-based
            #    semaphores in scheduling order, so emission order
            #    tracks completion order.
            xh_t = [None] * len(XBLOCKS)
            rh_t = [None] * len(XBLOCKS)
            for bi, (s, n) in enumerate(XBLOCKS):
                if bi < 3 or bi == len(XBLOCKS) - 1:
                    xh_t[bi] = pp.tile([P, n, T_PC], f16, name=f"xhp{bi}")
                else:
                    xh_t[bi] = xpool.tile(
                        [P, n, T_PC], f16, tag="xh4", name=f"xh{bi}"
                    )
                if bi == 2:
                    eng = nc.scalar
                elif bi == len(XBLOCKS) - 1:
                    eng = nc.scalar
                else:
                    eng = nc.sync if bi % 2 == 1 else nc.gpsimd
                eng.dma_start(xh_t[bi][:], xh_v[:, s : s + n, :])
                if bi in (3, 5, 7):
                    c = ARC * (bi - 1) // 2
                    nc.gpsimd.dma_start(
                        arsb[:, c : c + ARC, :], ar_v[:, c : c + ARC, :]
                    )
            for bi, (s, n) in enumerate(XBLOCKS):
                if bi < 3 or bi == len(XBLOCKS) - 1:
                    rh_t[bi] = pp.tile([P, n, T_PC], f16, name=f"rhp{bi}")
                else:
                    rh_t[bi] = rpool.tile(
                        [P, n, T_PC], f16, tag="rh4", name=f"rh{bi}"
                    )
                if bi == 2 or bi == len(XBLOCKS) - 1:
                    eng = nc.scalar
                else:
                    eng = nc.sync if bi % 2 == 1 else nc.gpsimd
                eng.dma_start(rh_t[bi][:], rh_v[:, s : s + n, :])
                if bi == 0:
                    nc.gpsimd.dma_start(e8sb[:], e8[:])
                    nc.scalar.dma_start(idsb[:], idm[:])
                elif bi == 3:
                    wload(2, nc.gpsimd)
                elif bi == 5:
                    wload(3, nc.sync)
            # tails
            nc.sync.dma_start(lbsb[:], lb[:])
            nc.gpsimd.dma_start(bbsb[:], bb_v[:])
            wload(4, nc.sync)
            wload(5, nc.gpsimd)

            # ---- pools; creation order = reverse close order ----
            phase1 = ExitStack()
            gp = phase1.enter_context(tc.tile_pool(name="gtmp", bufs=2))
            ps_h = phase1.enter_context(
                tc.tile_pool(name="ps_h", bufs=2, space="PSUM")
            )
            stack_e = ExitStack()
            ps_e = stack_e.enter_context(
                tc.tile_pool(name="ps_e", bufs=3, space="PSUM")
            )
            stack_l = ExitStack()
            ps_l = stack_l.enter_context(
                tc.tile_pool(name="ps_l", bufs=NGT, space="PSUM")
            )

            lgs_t = [
                ps_l.tile([2 * E, GT], f32, tag="lg", name=f"lg{g}")
                for g in range(NGT)
            ]
            h_t = [
                ps_h.tile([P, TTILE], f32, tag="h", name=f"h{t}")
                for t in range(NT)
            ]

            # base-only group: 16 DR steps closing WITHOUT the lora
            # term; epilogue (x1/64 + bias) goes to an SBUF staging
            # tile so the PSUM bank frees immediately
            osb_b = {}

            def base_group_open(acc, ot, tt, k2s):
                ts = slice(tt * TTILE, (tt + 1) * TTILE)
                for k2 in k2s:
                    nc.tensor.matmul(
                        acc[:],
                        lhsT=w_sb[ot][:, k2, :, :],
                        rhs=x8sb[:, 2 * k2 : 2 * k2 + 2, ts],
                        start=(k2 == 0),
                        stop=(k2 == KO2 - 1),
                        perf_mode=DR,
                    )

            def base_group_close(acc, ot, tt):
                t_ = obb.tile(
                    [P, TTILE], f16, tag="ob", name=f"ob{ot}_{tt}"
                )
                nc.vector.scalar_tensor_tensor(
                    t_[:],
                    acc[:],
                    1.0 / W8_SCALE,
                    bbsb[:, ot, None].to_broadcast((P, TTILE)),
                    Alu.mult,
                    Alu.add,
                )
                osb_b[(ot, tt)] = t_

            # ---- phase A: xh stream -> router-hi + h + casts + ot0 ----
            for ko in range(KO):
                bi, off = KOBLK[ko]
                xh_ko = xh_t[bi][:, off, :]
                for g in range(NGT):
                    gs = slice(g * GT, (g + 1) * GT)
                    if ko < KO - 1:   # t13(31) held back to close lg
                        nc.tensor.matmul(
                            lgs_t[g][:],
                            lhsT=rw2sb[:, ko, :],
                            rhs=xh_ko[:, gs],
                            start=(ko == 0),
                            stop=False,
                        )
                for tt in range(NT):
                    ts = slice(tt * TTILE, (tt + 1) * TTILE)
                    nc.tensor.matmul(
                        h_t[tt][:],
                        lhsT=arsb[:, ko, :],
                        rhs=xh_ko[:, ts],
                        start=(ko == 0),
                        stop=(ko == KO - 1),
                    )
                if ko % 2 == 0:
                    nc.vector.tensor_copy(x8sb[:, ko, :], xh_ko[:])
                else:
                    nc.scalar.activation(x8sb[:, ko, :], xh_ko[:], Act.Copy)

            # ---- phase B: rh stream -> router-lo + ot1..3 base-only ----
            # interleave: per rh-ko the two t2 terms plus up to 4 DR
            # steps from the ot1..3 group queue
            bq = []
            for ot in (0, 1, 2, 3):
                for tt in range(2):
                    bq.append((ot, tt))
            bq_steps = [
                (ot, tt, k2) for ot, tt in bq for k2 in range(KO2)
            ]
            bqi = 0
            bacc = {}

            def bq_emit(n):
                nonlocal bqi
                for _ in range(n):
                    if bqi >= len(bq_steps):
                        return
                    ot, tt, k2 = bq_steps[bqi]
                    bqi += 1
                    if k2 == 0:
                        bacc[(ot, tt)] = ps_e.tile(
                            [P, TTILE], f32, tag="acce", name=f"e{ot}{tt}"
                        )
                    ts = slice(tt * TTILE, (tt + 1) * TTILE)
                    nc.tensor.matmul(
                        bacc[(ot, tt)][:],
                        lhsT=w_sb[ot][:, k2, :, :],
                        rhs=x8sb[:, 2 * k2 : 2 * k2 + 2, ts],
                        start=(k2 == 0),
                        stop=(k2 == KO2 - 1),
                        perf_mode=DR,
                    )
                    if k2 == KO2 - 1:
                        base_group_close(bacc.pop((ot, tt)), ot, tt)

            for ko in range(KO):
                bi, off = KOBLK[ko]
                rh_ko = rh_t[bi][:, off, :]
                # a full group as prologue covers the first rh block's
                # arrival latency; then 3 filler steps per rh-ko
                bq_emit(16 if ko == 0 else 4)
                for g in range(NGT):
                    gs = slice(g * GT, (g + 1) * GT)
                    nc.tensor.matmul(
                        lgs_t[g][:E, :],
                        lhsT=rw2sb[:, ko, :E],
                        rhs=rh_ko[:, gs],
                        start=False,
                        stop=False,
                    )
            # drain any leftover queued steps
            bq_emit(len(bq_steps))
            # close the logits accumulation: the held-back t13(31)
            ko = KO - 1
            biL, offL = KOBLK[ko]
            for g in range(NGT):
                gs = slice(g * GT, (g + 1) * GT)
                nc.tensor.matmul(
                    lgs_t[g][:],
                    lhsT=rw2sb[:, ko, :],
                    rhs=xh_t[biL][:, offL, gs],
                    start=False,
                    stop=(g == NGT - 1) or True,
                )

            # ---- gating (DVE) overlapped with ot4 base-only (PE) ----
            lgs16 = []
            for g in range(NGT):
                t_ = gp.tile([2 * E, GT], f32, tag="lgs", name=f"lgs{g}")
                nc.vector.tensor_copy(t_[:], lgs_t[g][:])
                lgs16.append(t_)
            stack_l.close()
            stack_t = ExitStack()
            ps_t = stack_t.enter_context(
                tc.tile_pool(name="ps_t", bufs=2, space="PSUM")
            )

            # token-major transpose of the [16, GT] logits (both tiles)
            ltk16s = []
            for g in range(NGT):
                ltk16 = gp.tile([P, NGC, 2 * E], f32, tag="ltk16", name=f"lt16{g}")
                for c in range(NGC):
                    tp = ps_t.tile([P, GT], f32, tag="pt", name="tp")[:, : 2 * E]
                    nc.tensor.transpose(
                        tp[:], lgs16[g][:, c * P : (c + 1) * P],
                        idsb[: 2 * E, : 2 * E],
                    )
                    nc.vector.tensor_copy(ltk16[:, c, :], tp[:])
                ltk16s.append(ltk16)

            # PE filler while the DVE top-2 chain runs
            acc40 = ps_e.tile([P, TTILE], f32, tag="acce", name="e40")
            base_group_open(acc40, 4, 0, range(KO2))

            gates = []
            for g in range(NGT):
                ltk16 = ltk16s[g]
                ltk = gp.tile([P, NGC, E], f32, tag="ltk", name=f"ltk{g}")
                nc.vector.tensor_tensor(
                    ltk[:], ltk16[:, :, :E], ltk16[:, :, E:], Alu.add
                )
                m1 = gp.tile([P, NGC, 1], f32, tag="m1")
                nc.vector.tensor_reduce(m1[:], ltk[:], mybir.AxisListType.X, Alu.max)
                mask1 = gp.tile([P, NGC, E], f32, tag="mask1")
                nc.vector.tensor_tensor(
                    mask1[:], ltk[:], m1.to_broadcast((P, NGC, E)), Alu.is_equal
                )
                l2 = gp.tile([P, NGC, E], f32, tag="l2")
                nc.vector.scalar_tensor_tensor(
                    l2[:], mask1[:], -1e30, ltk[:], Alu.mult, Alu.add
                )
                m2 = gp.tile([P, NGC, 1], f32, tag="m2")
                nc.vector.tensor_reduce(m2[:], l2[:], mybir.AxisListType.X, Alu.max)
                mask2 = gp.tile([P, NGC, E], f32, tag="mask2")
                nc.vector.tensor_tensor(
                    mask2[:], l2[:], m2.to_broadcast((P, NGC, E)), Alu.is_equal
                )
                dlt = gp.tile([P, NGC, 1], f32, tag="dlt")
                nc.vector.tensor_tensor(dlt[:], m2[:], m1[:], Alu.subtract)
                dlts = gp.tile([P, NGC, 1], f32, tag="dlts")
                nc.vector.tensor_scalar(
                    dlts[:], dlt[:], 1.0 / 64.0, 0.0, Alu.mult, Alu.add
                )
                g2 = gp.tile([P, NGC, 1], f32, tag="g2")
                nc.scalar.activation(g2[:], dlts[:], Act.Sigmoid)
                g1 = gp.tile([P, NGC, 1], f32, tag="g1")
                nc.vector.tensor_scalar(g1[:], g2[:], -1.0, 1.0, Alu.mult, Alu.add)

                gate = gp.tile([P, NGC, E], f32, tag="gate", name=f"gate{g}")
                nc.vector.tensor_tensor(
                    gate[:], mask1[:], g1.to_broadcast((P, NGC, E)), Alu.mult
                )
                gm2 = gp.tile([P, NGC, E], f32, tag="gm2")
                nc.vector.tensor_tensor(
                    gm2[:], mask2[:], g2.to_broadcast((P, NGC, E)), Alu.mult
                )
                nc.vector.tensor_tensor(gate[:], gate[:], gm2[:], Alu.add)
                gates.append(gate)

            acc41 = ps_e.tile([P, TTILE], f32, tag="acce", name="e41")
            base_group_open(acc41, 4, 1, range(KO2))

            for g in range(NGT):
                gs = slice(g * GT, (g + 1) * GT)
                gts = gp.tile([E, GT], f32, tag="gts", name=f"gts{g}")
                for c in range(NGC):
                    tp2 = ps_t.tile([P, GT], f32, tag="pt", name="tp2")[:E, :P]
                    nc.tensor.transpose(tp2[:], gates[g][:, c, :], idsb[:])
                    nc.vector.tensor_copy(gts[:, c * P : (c + 1) * P], tp2[:])

                RG = ps_t.tile([P, GT], f32, tag="pt", name="RG")
                nc.tensor.matmul(
                    RG[:], lhsT=e8sb[:], rhs=gts[:], start=True, stop=True
                )
                nc.vector.tensor_copy(rgp[:, gs], RG[:])
                nc.vector.tensor_tensor(
                    hwsb[:, gs], h_t[g][:], rgp[:, gs], Alu.mult
                )

            base_group_close(acc40, 4, 0)
            base_group_close(acc41, 4, 1)

            stack_t.close()
            stack_e.close()
            phase1.close()

            # ---- phase C: fused groups ot5.. + lora passes for 0..4 ----
            phase2 = ExitStack()
            ps_o = phase2.enter_context(
                tc.tile_pool(name="ps_o", bufs=6, space="PSUM")
            )

            def lora_pass(ot, tt):
                """Add the gated-lora term to a staged base-only tile."""
                os_ = slice(ot * P, (ot + 1) * P)
                ts = slice(tt * TTILE, (tt + 1) * TTILE)
                acc = ps_o.tile([P, TTILE], f32, tag="acc", name="accl")
                nc.tensor.matmul(
                    acc[:],
                    lhsT=lbsb[:, os_],
                    rhs=hwsb[:, ts],
                    start=True,
                    stop=True,
                )
                osb = ob.tile([P, TTILE], f16, tag="osb", name="osbl")
                nc.vector.scalar_tensor_tensor(
                    osb[:],
                    acc[:],
                    1.0 / W8_SCALE,
                    osb_b.pop((ot, tt))[:],
                    Alu.mult,
                    Alu.add,
                )
                (nc.sync if tt == 0 else nc.scalar).dma_start(
                    yt_v[:, ot, ts], osb[:]
                )

            lq = [(ot, tt) for ot in range(N_SPLIT) for tt in range(2)]
            for ot in range(N_SPLIT, OTILES):
                for pre in (ot + 1, ot + 2, ot + 3):
                    if pre < OTILES and w_sb[pre] is None:
                        wload(pre, nc.scalar if pre % 2 == 0 else nc.gpsimd)
                os_ = slice(ot * P, (ot + 1) * P)
                for tt in range(NT):
                    ts = slice(tt * TTILE, (tt + 1) * TTILE)
                    acc = ps_o.tile([P, TTILE], f32, tag="acc")
                    for k2 in range(KO2):
                        nc.tensor.matmul(
                            acc[:],
                            lhsT=w_sb[ot][:, k2, :, :],
                            rhs=x8sb[:, 2 * k2 : 2 * k2 + 2, ts],
                            start=(k2 == 0),
                            stop=False,
                            perf_mode=DR,
                        )
                    nc.tensor.matmul(
                        acc[:],
                        lhsT=lbsb[:, os_],
                        rhs=hwsb[:, ts],
                        start=False,
                        stop=True,
                    )
                    osb = ob.tile([P, TTILE], f16, tag="osb")
                    nc.vector.scalar_tensor_tensor(
                        osb[:],
                        acc[:],
                        1.0 / W8_SCALE,
                        bbsb[:, ot, None].to_broadcast((P, TTILE)),
                        Alu.mult,
                        Alu.add,
                    )
                    (nc.sync if tt == 0 else nc.scalar).dma_start(
                        yt_v[:, ot, ts], osb[:]
                    )
                # two lora passes interleaved per fused group pair
                for _ in range(2):
                    if lq:
                        lora_pass(*lq.pop(0))
            while lq:
                lora_pass(*lq.pop(0))
            phase2.close()

    nc.compile()
    return nc


def get_program():
    if "nc" not in _prog_cache:
        _prog_cache["nc"] = _build_program()
    return _prog_cache["nc"]


def make_in_maps(x, base_w, base_b, lora_A, lora_B, router_w, scalings):
    """Host-side sharding/layout prep -> per-core input dicts."""
    import ml_dtypes

    x = np.ascontiguousarray(x, dtype=np.float32)
    # partition-major layout [P, KO, T]: per-core 4-ko DMA blocks are
    # 1MB with 8KB contiguous runs per partition
    xt_full = np.ascontiguousarray(
        x.reshape(T, KO, P).transpose(2, 1, 0)
    )  # [P, KO, T]

    # base weights x64 -> e4m3, DoubleRow pair layout [ot,p,k2,j,f]
    wt_host = np.ascontiguousarray(
        (base_w.T.astype(np.float32) * W8_SCALE)
        .reshape(KO2, 2, P, OTILES, P)
        .transpose(3, 2, 0, 1, 4)
        .reshape(OTILES * P, KO2 * 2 * P)
        .astype(ml_dtypes.float8_e4m3)
    )
    lb_host = np.ascontiguousarray(
        lora_B.T.astype(np.float32).astype(ml_dtypes.bfloat16)
    )

    # lora_A.T (unscaled; scaling folded into e8) -> [p, ko*128+r]
    ar_host = np.ascontiguousarray(
        lora_A.T.astype(np.float32)
        .reshape(KO, P, R)
        .transpose(1, 0, 2)
        .reshape(P, KO * R)
        .astype(np.float16)
    )

    # router_w.T x64 -> [p, ko, 16]: cols 0:8 = fp16 hi, 8:16 = fp16 lo
    # (hi + lo == 64*rw to ~2^-24 relative)
    rw64 = np.ascontiguousarray(
        router_w.T.astype(np.float32)
        .reshape(KO, P, E)
        .transpose(1, 0, 2)
    ) * np.float32(64.0)                                  # [P, KO, E]
    rwh_host = rw64.astype(np.float16)
    rwr_host = (rw64 - rwh_host.astype(np.float32)).astype(np.float16)
    rw2_host = np.ascontiguousarray(
        np.concatenate([rwh_host, rwr_host], axis=-1).reshape(P, KO * 2 * E)
    )

    # expert -> rank-slot expansion with per-expert scaling and the x64
    # fp8 weight scale folded in (so the lora matmul accumulates at the
    # same scale as the fp8 base steps)
    e8 = np.zeros((E, P), dtype=np.float32)
    s = np.asarray(scalings, dtype=np.float32) * W8_SCALE
    for e in range(E):
        e8[e, e * RANK : (e + 1) * RANK] = s[e]
    idm = np.eye(P, dtype=np.float32)
    bbf = base_b.astype(np.float32)

    xh_full = xt_full.astype(np.float16)
    rh_full = (xt_full - xh_full.astype(np.float32)).astype(np.float16)

    in_maps = []
    for c in range(N_CORES):
        cs = slice(c * T_PC, (c + 1) * T_PC)
        m = {
            "xh": np.ascontiguousarray(xh_full[:, :, cs]).reshape(P, KO * T_PC),
            "rh": np.ascontiguousarray(rh_full[:, :, cs]).reshape(P, KO * T_PC),
            "wt": wt_host,
            "lb": lb_host,
            "ar": ar_host,
            "rw2": rw2_host,
            "bb": bbf,
            "e8": e8,
            "idm": idm,
        }
        in_maps.append(m)
    return in_maps


def assemble_output(results):
    """Per-core yt [O, T_PC] fp16 -> full [B, S, O] f32."""
    yt_full = np.concatenate(
        [np.asarray(r["yt"]) for r in results], axis=1
    )  # [O, T] fp16
    return np.ascontiguousarray(yt_full.T.astype(np.float32)).reshape(B, S, O)


def kernel(**inputs):
    _ensure_path()
    from concourse.bass_utils import run_bass_kernel_spmd

    assert int(inputs["top_k"]) == 2
    nc = get_program()
    in_maps = make_in_maps(
        inputs["x"],
        inputs["base_w"],
        inputs["base_b"],
        inputs["lora_A"],
        inputs["lora_B"],
        inputs["router_w"],
        inputs["scalings"],
    )
    res = run_bass_kernel_spmd(nc, in_maps, list(range(N_CORES)))
    return assemble_output(res.results)


if __name__ == "__main__":
    # quick smoke: build the program only
    get_program()
    print("program built OK")
